# revision 91
# baseline (speedup 1.0000x reference)
"""Trainium2 Bass kernel for NeuralClusteringAttention (v2, all-bf16).

Problem: B=4, T=1024, D=512, C=8 clusters, H=8 heads, fp32 reference.
Reference collapses to ONE masked-attention pass (each token belongs to
exactly one cluster; in_proj bias is zero):
    out_i = Wout @ (sum_{j in c_i} e^{s_ij} v_j) / (sum_{j in c_i} e^{s_ij}
            + (T - n_{c_i})) + bout,   s_ij = q_i.k_j / 8

Design (55.1us baseline -> 38.1us):
- Tokens sorted by cluster into canonical slots (width = max over batches +1,
  even; slot order tuned for pipeline ramp/drain). Attention runs on EXACT
  per-slot windows: query chunks are slot sub-ranges, key blocks <=128-wide
  sub-windows of the same slot at arbitrary column offsets. No cross-cluster
  coverage -> no mask rows (contraction 64), ~2.3x less score/exp/PV work
  than block-aligned coverage.
- Softmax denominator: pad columns inside a slot have k=v=0 so they add
  exp(0)=1 each; the slot's first pad column carries M = T - w + 1 in its
  v_sb ones-columns (Pool-engine memset patch), making the total exactly
  sum e^s + (T - n) with no extra compute pass.
- Everything bf16 on the PE (1.0 cycles/row at ANY moving width enables the
  exact-fit narrow matmuls; fp8 empirically fails the 2e-2 gate). bf16
  halves input DMA; output stays fp32 through an SBUF bounce (PSUM cannot
  be DMA'd). Few, large DMAs: each dma_start costs ~630ns serialized on the
  single HWDGE issue device.
- Normalization: DVE reciprocal of the replicated mass rows, then two
  strided-head tensor_muls per qsub writing the [hd-pair, hp] o_sc layout
  the 256-contraction output projection wants.
- Software-pipelined emission (engines execute in order): scores/exp run
  TRAIL=3 units ahead of PV/normalize; out-projection chunks trail their
  covering slots; six zero matmuls warm the PE p-state during the first
  DMA wave; wave 1 is one host-packed [xt-cols-0:128 | q/k-weights] tensor
  (one ~630ns HWDGE issue instead of two); a 44-col trailing qsub on the
  last slot shortens the pipeline drain.
- HW constraints found the hard way: tile_position must not change inside a
  PSUM accumulation group (scores are grouped by base-partition parity with
  each parity group in its own bank); one start=True per psum-bank lifetime
  (a second start zeroes the whole bank); SBUF APs start only at partition
  0/32/64/96; Pool/GPSIMD cannot touch PSUM.

Sharding: 8 cores = (4 batches) x (2 head-halves of 4 heads each); host
sums the two partial output projections per batch and adds bout.
"""

import ml_dtypes
import numpy as np

import concourse.bacc as bacc
import concourse.bass as bass
import concourse.mybir as mybir
import concourse.tile as tile
from concourse.bass_utils import run_bass_kernel_spmd

B, T, D, C, H = 4, 1024, 512, 8, 8
HD = D // H          # 64
LH = 4               # local heads per core
BF = mybir.dt.bfloat16
F32 = mybir.dt.float32
NPBF = ml_dtypes.bfloat16


def make_schedule(assign_all):
    """Canonical slot layout shared by all batches (single SPMD program).

    Returns (T2, slots, chunks, order, W):
      slots: per slot dict(a, w, kbs=[(k0, nk)..], qs=[(q0, qw)..], M)
      chunks: [(c0, c1)] qk-projection column chunks (pairs of slots)
    """
    counts = np.stack([np.bincount(a, minlength=C) for a in assign_all])
    order = np.argsort(-counts, axis=1, kind="stable")   # [B, C] rank->cluster
    sizes = -np.sort(-counts, axis=1)
    W = sizes.max(axis=0) + 1                            # >=1 pad col per slot
    W = (W + 1) // 2 * 2                                 # even widths
    # processing order = column order: ramp in on a mid slot, biggest slots
    # mid-stream, smallest last (short pipeline drain)
    perm = [int(p) for p in np.argsort(-W, kind="stable")]
    perm = perm[4:5] + perm[0:4] + perm[5:]
    W = W[perm]
    order = order[:, perm]
    A = np.zeros(C + 1, np.int64)
    A[1:] = np.cumsum(W)
    T2 = int(A[-1])

    slots = []
    for r in range(C):
        a, w = int(A[r]), int(W[r])
        kbs = [(a, min(128, w))]
        if w > 128:
            kbs.append((a + 128, w - 128))
        T2 = max(T2, a + 128, a + w)
        if w > 128:
            h1 = (w // 2 + 1) // 2 * 2                   # balanced even split
            qs = [(a, h1), (a + h1, w - h1)]
        elif r == C - 1:
            qs = [(a, w - 44), (a + w - 44, 44)]         # short drain chain
        else:
            qs = [(a, w)]
        slots.append(dict(a=a, w=w, kbs=kbs, qs=qs, M=float(T - w + 1)))
    chunks = [(int(A[i]), int(A[min(i + 2, C)])) for i in range(0, C, 2)]
    chunks[-1] = (chunks[-1][0], T2)
    # out-projection chunks: flat 128-grid over the used columns (bank-exact
    # PSUM tiles, 512B DMA elems), independent of slot boundaries
    used = int(A[-1])
    ochunks = [(g, min(g + 128, used) - g) for g in range(0, used, 128)]
    return T2, slots, chunks, ochunks, order, W


def _hashable(slots):
    return tuple(
        (s["a"], s["w"], tuple(s["kbs"]), tuple(s["qs"]), s["M"])
        for s in slots
    )


def _kernel_body(tc, T2, slots, chunks, ochunks):
    nc = tc.nc
    NKB = sum(len(s["kbs"]) for s in slots)

    xt_d = nc.dram_tensor("xt", [D, T2], BF, kind="ExternalInput").ap()
    w1_d = nc.dram_tensor("w1", [D, 640], BF, kind="ExternalInput").ap()
    wqkv_d = nc.dram_tensor("wqkv", [D, 768], BF, kind="ExternalInput").ap()
    wo_d = nc.dram_tensor("wo", [256, D], BF, kind="ExternalInput").ap()
    out_d = nc.dram_tensor("outT", [D, T2], F32, kind="ExternalOutput").ap()
    out_r = out_d.rearrange("(c p) t -> p c t", p=128)

    with (
        tc.tile_pool(name="const", bufs=1) as const,
        tc.tile_pool(name="p1", bufs=4) as p1pool,
        tc.tile_pool(name="rb", bufs=4) as rbpool,
        tc.tile_pool(name="psum", bufs=1, space="PSUM") as psum,
    ):
        # ---- persistent SBUF tiles + input DMAs ----
        # first pieces sized for the earliest possible first matmul: q/k
        # weights, then xt chunk 0, then the rest
        wqkv = const.tile([128, D // 128, 768], BF)
        wqkv_r = wqkv_d.rearrange("(c p) n -> p c n", p=128)
        xt = const.tile([128, D // 128, T2], BF)
        xt_r = xt_d.rearrange("(c p) t -> p c t", p=128)
        # DMA waves, smallest-first. Wave 1 is ONE transfer of a host-packed
        # tensor [xt cols 0:128 | q/k weights] so the first projection piece
        # starts ~1us earlier (each dma_start serializes ~630ns on the single
        # HWDGE issue device, and transfers serialize on DMA_ENGINES)
        w1 = const.tile([128, D // 128, 640], BF)
        w1_r = w1_d.rearrange("(c p) n -> p c n", p=128)
        nc.sync.dma_start(w1[:], w1_r)
        nc.scalar.dma_start(xt[:, :, 128:512], xt_r[:, :, 128:512])
        nc.sync.dma_start(wqkv[:, :, 512:768], wqkv_r[:, :, 512:768])
        nc.scalar.dma_start(xt[:, :, 512:T2], xt_r[:, :, 512:T2])
        wo = const.tile([128, 2, D], BF)
        nc.sync.dma_start(wo[:], wo_d.rearrange("(c p) n -> p c n", p=128))

        def xtc(dc, c0, c1):
            # xt columns 0:128 live only in the packed wave-1 tile
            if c1 <= 128:
                return w1[:, dc, c0:c1]
            assert c0 >= 128
            return xt[:, dc, c0:c1]

        def wqk(dc, c0, c1):
            # q/k weight cols 0:512 live only in the packed wave-1 tile
            assert c1 <= 512
            return w1[:, dc, 128 + c0:128 + c1]

        warm = const.tile([128, 512], BF)
        nc.vector.memset(warm, 0.0)
        wps = psum.tile([128, 512], F32, tag="misc", bufs=2, name="wps")
        for wi in range(6):
            nc.tensor.matmul(wps[:, :512], lhsT=warm[:, 0:128], rhs=warm,
                             start=(wi == 0), stop=(wi == 5))

        qk2 = const.tile([128, 2, 2, T2], BF)    # [(q|k), hp, col]
        o_sc = const.tile([128, 2, T2], BF)
        # V augmented with ones-columns (softmax mass lands on psum rows
        # 64:128); the denominator pad column of each slot carries M instead.
        v_sb = const.tile([128, NKB, LH, 128], BF)
        nc.gpsimd.memset(v_sb[:, :, :, 64:128], 1.0)
        kbi = 0
        kb_of_slot = []
        for s in slots:
            kb_of_slot.append(kbi)
            # denominator pad column = FIRST column of the slot (partition 0
            # of kb0 — SBUF APs may only start at partition 0/32/64/96);
            # SBUF-only writes can go on the otherwise-idle Pool engine
            nc.gpsimd.memset(v_sb[0:1, kbi, :, 64:128], s["M"])
            kbi += len(s["kbs"])

        # ---- per-chunk: qk projection, then per-slot v + attention ----
        # ---- emission helpers (software-pipelined schedule below) ----
        ei = [0]

        def alt():
            ei[0] += 1
            return (nc.vector.tensor_copy, nc.scalar.copy)[ei[0] % 2]

        piece_bounds = [0, 128] + list(range(384, T2, 256)) + [T2]
        pieces_done = [False] * (len(piece_bounds) - 1)

        def qkproj_piece(pi_):
            """q+k projection for one <=256-col piece; fused 1-bank psum."""
            c0, c1 = piece_bounds[pi_], piece_bounds[pi_ + 1]
            W_ = c1 - c0
            for hp in range(2):
                ps = psum.tile([128, 2, 256], F32, tag="misc", bufs=2,
                               name=f"pp{pi_}_{hp}")
                for qk in range(2):
                    # q and k halves share one psum bank: a second start=True
                    # would zero the bank on HW and wipe the q result, so the
                    # whole (qk, dc) sweep is ONE accumulation group
                    for dc in range(D // 128):
                        nc.tensor.matmul(
                            ps[:, qk, :W_],
                            lhsT=wqk(dc, hp * 256 + qk * 128,
                                     hp * 256 + (qk + 1) * 128),
                            rhs=xtc(dc, c0, c1),
                            start=(qk == 0 and dc == 0),
                            stop=(qk == 1 and dc == D // 128 - 1),
                        )
                alt()(qk2[:, :, hp, c0:c1], ps[:, :, :W_])

        def need_pieces(col_end):
            col_end = min(col_end, T2)
            for pi_ in range(len(pieces_done)):
                if piece_bounds[pi_] < col_end and not pieces_done[pi_]:
                    pieces_done[pi_] = True
                    qkproj_piece(pi_)

        def vproj(r):
            """v projection for one slot; kb windows extended to 128 so a
            single full copy is fully initialized (extra rows unused)."""
            s = slots[r]
            nkb = len(s["kbs"])
            psv = psum.tile([128, 2, 256], F32, tag="misc", bufs=2,
                            name=f"psv{r}")
            for j, (k0, nk) in enumerate(s["kbs"]):
                kw = min(128, T2 - k0)
                for dc in range(D // 128):
                    # the two kb halves share a psum bank: one accumulation
                    # group (start zeroes the bank once, up front)
                    nc.tensor.matmul(
                        psv[:kw, j, :],
                        lhsT=xtc(dc, k0, k0 + kw),
                        rhs=wqkv[:, dc, 512:768],
                        start=(dc == 0 and j == 0),
                        stop=(dc == D // 128 - 1 and j == len(s["kbs"]) - 1),
                    )
            kb0 = kb_of_slot[r]
            alt()(v_sb[:, kb0:kb0 + nkb, :, 0:64],
                  psv[:, :nkb, :].rearrange("p k (h d) -> p k h d", h=LH))

        def stage_a(i):
            """scores + exp for qsub unit i (all key blocks, kb j at column
            offset 128j of one 2-bank tile).

            tile_position may not change inside a PSUM accumulation group on
            HW, so heads are grouped by base-partition parity with each
            parity group in its OWN bank (par stride = 2KB)."""
            r, (q0, qw) = units[i]
            kbs = slots[r]["kbs"]
            s_ps = psum.tile([128, 2, 2, 256], F32, tag="sps", bufs=2,
                             name=f"sps{i}")
            for par in range(2):
                # key windows extended to 128 (cost scales with the moving
                # dim only) so s_ps is fully initialized; rows nk:128 unused
                pb = 64 * par
                for j, (k0, nk) in enumerate(kbs):
                    for hi, h in enumerate((par, 2 + par)):
                        nc.tensor.matmul(
                            s_ps[:, par, hi, 128 * j:128 * j + qw],
                            lhsT=qk2[pb:pb + 64, 1, h // 2, k0:k0 + 128],
                            rhs=qk2[pb:pb + 64, 0, h // 2, q0:q0 + qw],
                            start=(j == 0 and hi == 0),
                            stop=(j == len(kbs) - 1 and hi == 1),
                        )
            p1 = p1pool.tile([128, 2, 2, 256], BF, tag="p1", bufs=4,
                             name=f"p1_{i}")
            # single exp per qsub via a 5-dim AP: (par, hi, kb, q)
            s5 = s_ps.rearrange("p a b (c q) -> p a b c q", c=2)
            p5 = p1.rearrange("p a b (c q) -> p a b c q", c=2)
            nc.scalar.activation(
                p5[:, :, :, 0:len(kbs), :qw], s5[:, :, :, 0:len(kbs), :qw],
                mybir.ActivationFunctionType.Exp, scale=0.125,
            )
            return p1

        def stage_b(i, p1s_i):
            """PV + recip + normalize-muls for qsub unit i."""
            r, (q0, qw) = units[i]
            s = slots[r]
            kb0 = kb_of_slot[r]
            nkb = len(s["kbs"])
            oaug = psum.tile([128, LH, 128], F32, tag="oaug", bufs=2,
                             name=f"oaug{i}")
            for j, (k0, nk) in enumerate(s["kbs"]):
                for h in range(LH):
                    nc.tensor.matmul(
                        oaug[:, h, :qw],
                        lhsT=v_sb[0:nk, kb0 + j, h, :],
                        rhs=p1s_i[:nk, h % 2, h // 2, 128 * j:128 * j + qw],
                        start=(j == 0 and h == 0),
                        stop=(j == nkb - 1 and h == LH - 1),
                    )
            recip = rbpool.tile([64, LH, 128], F32, tag="rb", name=f"rc{i}")
            nc.vector.reciprocal(recip[:, :, :qw], oaug[64:128, :, :qw])
            # two muls via strided head reads: even heads land on partitions
            # 0:64 of both head-pair planes, odd heads on 64:128 — exactly the
            # [hd-pair, hp] layout the 256-contraction out-proj wants
            for par in range(2):
                nc.vector.tensor_mul(
                    o_sc[64 * par:64 * par + 64, :, q0:q0 + qw],
                    oaug[0:64, par::2, :qw],
                    recip[:, par::2, :qw],
                )

        def stage_c(oi):
            """output projection + store for one flat 128-grid chunk."""
            g0, gw = ochunks[oi]
            po = psum.tile([128, 4, 128], F32, tag="misc", bufs=2,
                           name=f"po{oi}")
            for doc in range(4):
                for dhc in range(2):
                    nc.tensor.matmul(
                        po[:, doc, :gw],
                        lhsT=wo[:, dhc, doc * 128:(doc + 1) * 128],
                        rhs=o_sc[:, dhc, g0:g0 + gw],
                        start=(doc == 0 and dhc == 0),
                        stop=(doc == 3 and dhc == 1),
                    )
            ost = rbpool.tile([128, 4, 128], F32, tag="ost", name=f"ost{oi}")
            nc.scalar.copy(ost[:, :, :gw], po[:, :, :gw])
            nc.sync.dma_start(out_r[:, :, g0:g0 + gw], ost[:, :, :gw])

        # flat qsub unit list; vproj(r) runs with the LAST qsub of slot r
        # (its PV is still >=1 stage away in the software pipeline)
        units = []
        pre = []   # slot index whose vproj runs before this unit, or None
        for r, s in enumerate(slots):
            for qi, q in enumerate(s["qs"]):
                units.append((r, q))
                pre.append(r if qi == len(s["qs"]) - 1 else None)

        # software pipeline: A(i) one qsub ahead of B(i-1); out-proj grid
        # chunks trail the B covering them by one stage
        NU = len(units)
        p1s = [None] * NU
        last_of_slot = {}
        for i, (r, _) in enumerate(units):
            last_of_slot[r] = i
        cmax = []          # per ochunk: last unit index covering it
        for g0, gw in ochunks:
            cmax.append(max(i for i, (r, (q0, qw)) in enumerate(units)
                            if q0 < g0 + gw and q0 + qw > g0))
        bdone_at = {}
        cqi = 0
        bcount = 0

        def after_b(j):
            nonlocal bcount, cqi
            bcount += 1
            bdone_at[j] = bcount
            while cqi < len(ochunks) and (
                    bdone_at.get(cmax[cqi], 1 << 30)
                    + (0 if cmax[cqi] >= NU - 2 else 1) <= bcount):
                stage_c(cqi)
                cqi += 1

        TRAIL = 3
        for i in range(NU):
            r, (q0, qw) = units[i]
            s = slots[r]
            # key windows reach <=128 cols past each kb start
            need_pieces(max(k0 + 128 for k0, _ in s["kbs"]))
            if pre[i] is not None:
                vproj(pre[i])
            p1s[i] = stage_a(i)
            if i >= TRAIL:
                stage_b(i - TRAIL, p1s[i - TRAIL])
                p1s[i - TRAIL] = None
                after_b(i - TRAIL)
        for i in range(max(0, NU - TRAIL), NU):
            stage_b(i, p1s[i])
            after_b(i)
        while cqi < len(ochunks):
            stage_c(cqi)
            cqi += 1


def build_nc(T2, slots, chunks, ochunks):
    nc = bacc.Bacc("TRN2", target_bir_lowering=False, debug=False, num_devices=8)
    with tile.TileContext(nc) as tc:
        _kernel_body(tc, T2, slots, chunks, ochunks)
    nc.compile()
    return nc


def prepare(X, Wc, bc, Win, Wout):
    """Host-side clustering, canonical layout, and per-core input maps."""
    X = np.asarray(X, np.float32)
    Wc = np.asarray(Wc, np.float32)
    bc = np.asarray(bc, np.float32)
    Win = np.asarray(Win, np.float32)
    Wout = np.asarray(Wout, np.float32)

    assign_all = np.stack([(X[b] @ Wc.T + bc).argmax(-1) for b in range(B)])
    T2, slots, chunks, ochunks, order, W = make_schedule(assign_all)

    per_batch = []
    poss = []
    for b in range(B):
        a = assign_all[b]
        X2 = np.zeros((T2, D), np.float32)
        pos = np.empty(T, np.int64)
        for r in range(C):
            c = order[b, r]
            toks = np.nonzero(a == c)[0]
            n = len(toks)
            A0 = slots[r]["a"] + 1   # col A0-1 is the denominator pad column
            X2[A0:A0 + n] = X[b, toks]
            pos[toks] = np.arange(A0, A0 + n)
        per_batch.append({"xt": np.ascontiguousarray(X2.T).astype(NPBF)})
        poss.append(pos)

    per_half = []
    for hh in range(2):
        r = slice(hh * 256, (hh + 1) * 256)
        wq, wk, wv = Win[0:D][r].T, Win[D:2 * D][r].T, Win[2 * D:][r].T
        # [q-hp0 | k-hp0 | q-hp1 | k-hp1 | v]: hp0's q+k contiguous so the
        # first DMA wave is a single transfer
        wqkv = np.concatenate(
            [wq[:, 0:128], wk[:, 0:128], wq[:, 128:256], wk[:, 128:256], wv],
            axis=1,
        )
        per_half.append({
            "wqkv": np.ascontiguousarray(wqkv).astype(NPBF),
            "wo": np.ascontiguousarray(Wout[:, r].T).astype(NPBF),
        })

    in_maps = []
    for g in range(8):
        m = dict(per_batch[g // 2], **per_half[g % 2])
        # packed wave-1 tensor: [xt cols 0:128 | q/k weight cols 0:512]
        m["w1"] = np.ascontiguousarray(
            np.concatenate([m["xt"][:, 0:128], m["wqkv"][:, 0:512]], axis=1))
        in_maps.append(m)
    return (T2, slots, chunks, ochunks), in_maps, poss


_NC_CACHE = {}


def kernel(X, Wc, bc, Win, bin_, Wout, bout):
    assert not np.any(np.asarray(bin_)), "kernel assumes zero in_proj bias"
    sched, in_maps, poss = prepare(X, Wc, bc, Win, Wout)
    key = (sched[0], _hashable(sched[1]), tuple(sched[2]), tuple(sched[3]))
    if key not in _NC_CACHE:
        _NC_CACHE[key] = build_nc(*sched)
    nc = _NC_CACHE[key]
    res = run_bass_kernel_spmd(nc, in_maps, core_ids=list(range(8)))
    outs = res.results
    bout = np.asarray(bout, np.float32)
    out = np.empty((B, T, D), np.float32)
    for b in range(B):
        full = outs[2 * b]["outT"] + outs[2 * b + 1]["outT"]   # [D, T2]
        out[b] = full.T[poss[b]] + bout
    return out


# revision 94
# speedup vs baseline: 1.0032x; 1.0032x over previous
"""Trainium2 Bass kernel for NeuralClusteringAttention (v2, all-bf16).

Problem: B=4, T=1024, D=512, C=8 clusters, H=8 heads, fp32 reference.
Reference collapses to ONE masked-attention pass (each token belongs to
exactly one cluster; in_proj bias is zero):
    out_i = Wout @ (sum_{j in c_i} e^{s_ij} v_j) / (sum_{j in c_i} e^{s_ij}
            + (T - n_{c_i})) + bout,   s_ij = q_i.k_j / 8

Design (55.1us baseline -> 38.1us):
- Tokens sorted by cluster into canonical slots (width = max over batches +1,
  even; slot order tuned for pipeline ramp/drain). Attention runs on EXACT
  per-slot windows: query chunks are slot sub-ranges, key blocks <=128-wide
  sub-windows of the same slot at arbitrary column offsets. No cross-cluster
  coverage -> no mask rows (contraction 64), ~2.3x less score/exp/PV work
  than block-aligned coverage.
- Softmax denominator: pad columns inside a slot have k=v=0 so they add
  exp(0)=1 each; the slot's first pad column carries M = T - w + 1 in its
  v_sb ones-columns (Pool-engine memset patch), making the total exactly
  sum e^s + (T - n) with no extra compute pass.
- Everything bf16 on the PE (1.0 cycles/row at ANY moving width enables the
  exact-fit narrow matmuls; fp8 empirically fails the 2e-2 gate). bf16
  halves input DMA; output stays fp32 through an SBUF bounce (PSUM cannot
  be DMA'd). Few, large DMAs: each dma_start costs ~630ns serialized on the
  single HWDGE issue device.
- Normalization: DVE reciprocal of the replicated mass rows, then two
  strided-head tensor_muls per qsub writing the [hd-pair, hp] o_sc layout
  the 256-contraction output projection wants.
- Software-pipelined emission (engines execute in order): scores/exp run
  TRAIL=3 units ahead of PV/normalize; out-projection chunks trail their
  covering slots; six zero matmuls warm the PE p-state during the first
  DMA wave; wave 1 is one host-packed [xt-cols-0:128 | q/k-weights] tensor
  (one ~630ns HWDGE issue instead of two); a 44-col trailing qsub on the
  last slot shortens the pipeline drain.
- HW constraints found the hard way: tile_position must not change inside a
  PSUM accumulation group (scores are grouped by base-partition parity with
  each parity group in its own bank); one start=True per psum-bank lifetime
  (a second start zeroes the whole bank); SBUF APs start only at partition
  0/32/64/96; Pool/GPSIMD cannot touch PSUM.

Sharding: 8 cores = (4 batches) x (2 head-halves of 4 heads each); host
sums the two partial output projections per batch and adds bout.
"""

import ml_dtypes
import numpy as np

import concourse.bacc as bacc
import concourse.bass as bass
import concourse.mybir as mybir
import concourse.tile as tile
from concourse.bass_utils import run_bass_kernel_spmd

B, T, D, C, H = 4, 1024, 512, 8, 8
HD = D // H          # 64
LH = 4               # local heads per core
BF = mybir.dt.bfloat16
F32 = mybir.dt.float32
NPBF = ml_dtypes.bfloat16


def make_schedule(assign_all):
    """Canonical slot layout shared by all batches (single SPMD program).

    Returns (T2, slots, chunks, order, W):
      slots: per slot dict(a, w, kbs=[(k0, nk)..], qs=[(q0, qw)..], M)
      chunks: [(c0, c1)] qk-projection column chunks (pairs of slots)
    """
    counts = np.stack([np.bincount(a, minlength=C) for a in assign_all])
    order = np.argsort(-counts, axis=1, kind="stable")   # [B, C] rank->cluster
    sizes = -np.sort(-counts, axis=1)
    W = sizes.max(axis=0) + 1                            # >=1 pad col per slot
    W = (W + 1) // 2 * 2                                 # even widths
    # processing order = column order: ramp in on a mid slot, biggest slots
    # mid-stream, smallest last (short pipeline drain)
    perm = [int(p) for p in np.argsort(-W, kind="stable")]
    perm = perm[4:5] + perm[0:4] + perm[5:]
    W = W[perm]
    order = order[:, perm]
    A = np.zeros(C + 1, np.int64)
    A[1:] = np.cumsum(W)
    T2 = int(A[-1])

    slots = []
    for r in range(C):
        a, w = int(A[r]), int(W[r])
        kbs = [(a, min(128, w))]
        if w > 128:
            kbs.append((a + 128, w - 128))
        T2 = max(T2, a + 128, a + w)
        if w > 128:
            h1 = (w // 2 + 1) // 2 * 2                   # balanced even split
            qs = [(a, h1), (a + h1, w - h1)]
        elif r == C - 1:
            qs = [(a, w - 44), (a + w - 44, 44)]         # short drain chain
        else:
            qs = [(a, w)]
        slots.append(dict(a=a, w=w, kbs=kbs, qs=qs, M=float(T - w + 1)))
    chunks = [(int(A[i]), int(A[min(i + 2, C)])) for i in range(0, C, 2)]
    chunks[-1] = (chunks[-1][0], T2)
    # out-projection chunks: flat 128-grid over the used columns (bank-exact
    # PSUM tiles, 512B DMA elems), independent of slot boundaries
    used = int(A[-1])
    ochunks = [(g, min(g + 128, used) - g) for g in range(0, used, 128)]
    return T2, slots, chunks, ochunks, order, W


def _hashable(slots):
    return tuple(
        (s["a"], s["w"], tuple(s["kbs"]), tuple(s["qs"]), s["M"])
        for s in slots
    )


def _kernel_body(tc, T2, slots, chunks, ochunks):
    nc = tc.nc
    NKB = sum(len(s["kbs"]) for s in slots)

    xt_d = nc.dram_tensor("xt", [D, T2], BF, kind="ExternalInput").ap()
    w1_d = nc.dram_tensor("w1", [D, 640], BF, kind="ExternalInput").ap()
    wqkv_d = nc.dram_tensor("wqkv", [D, 768], BF, kind="ExternalInput").ap()
    wo_d = nc.dram_tensor("wo", [256, D], BF, kind="ExternalInput").ap()
    out_d = nc.dram_tensor("outT", [D, T2], F32, kind="ExternalOutput").ap()
    out_r = out_d.rearrange("(c p) t -> p c t", p=128)

    with (
        tc.tile_pool(name="const", bufs=1) as const,
        tc.tile_pool(name="p1", bufs=4) as p1pool,
        tc.tile_pool(name="rb", bufs=4) as rbpool,
        tc.tile_pool(name="psum", bufs=1, space="PSUM") as psum,
    ):
        # ---- persistent SBUF tiles + input DMAs ----
        # first pieces sized for the earliest possible first matmul: q/k
        # weights, then xt chunk 0, then the rest
        wqkv = const.tile([128, D // 128, 768], BF)
        wqkv_r = wqkv_d.rearrange("(c p) n -> p c n", p=128)
        xt = const.tile([128, D // 128, T2], BF)
        xt_r = xt_d.rearrange("(c p) t -> p c t", p=128)
        # DMA waves, smallest-first. Wave 1 is ONE transfer of a host-packed
        # tensor [xt cols 0:128 | q/k weights] so the first projection piece
        # starts ~1us earlier (each dma_start serializes ~630ns on the single
        # HWDGE issue device, and transfers serialize on DMA_ENGINES)
        w1 = const.tile([128, D // 128, 640], BF)
        w1_r = w1_d.rearrange("(c p) n -> p c n", p=128)
        nc.sync.dma_start(w1[:], w1_r)
        nc.scalar.dma_start(xt[:, :, 128:512], xt_r[:, :, 128:512])
        nc.sync.dma_start(wqkv[:, :, 512:768], wqkv_r[:, :, 512:768])
        nc.scalar.dma_start(xt[:, :, 512:T2], xt_r[:, :, 512:T2])
        wo = const.tile([128, 2, D], BF)
        nc.sync.dma_start(wo[:], wo_d.rearrange("(c p) n -> p c n", p=128))

        def xtc(dc, c0, c1):
            # xt columns 0:128 live only in the packed wave-1 tile
            if c1 <= 128:
                return w1[:, dc, c0:c1]
            assert c0 >= 128
            return xt[:, dc, c0:c1]

        def wqk(dc, c0, c1):
            # q/k weight cols 0:512 live only in the packed wave-1 tile
            assert c1 <= 512
            return w1[:, dc, 128 + c0:128 + c1]

        warm = const.tile([128, 512], BF)
        nc.vector.memset(warm, 0.0)
        wps = psum.tile([128, 512], F32, tag="misc", bufs=2, name="wps")
        for wi in range(6):
            nc.tensor.matmul(wps[:, :512], lhsT=warm[:, 0:128], rhs=warm,
                             start=(wi == 0), stop=(wi == 5))

        qk2 = const.tile([128, 2, 2, T2], BF)    # [(q|k), hp, col]
        o_sc = const.tile([128, 2, T2], BF)
        # V augmented with ones-columns (softmax mass lands on psum rows
        # 64:128); the denominator pad column of each slot carries M instead.
        v_sb = const.tile([128, NKB, LH, 128], BF)
        nc.gpsimd.memset(v_sb[:, :, :, 64:128], 1.0)
        kbi = 0
        kb_of_slot = []
        for s in slots:
            kb_of_slot.append(kbi)
            # denominator pad column = FIRST column of the slot (partition 0
            # of kb0 — SBUF APs may only start at partition 0/32/64/96);
            # SBUF-only writes can go on the otherwise-idle Pool engine
            nc.gpsimd.memset(v_sb[0:1, kbi, :, 64:128], s["M"])
            kbi += len(s["kbs"])

        # ---- per-chunk: qk projection, then per-slot v + attention ----
        # ---- emission helpers (software-pipelined schedule below) ----
        ei = [0]

        def alt():
            ei[0] += 1
            return (nc.vector.tensor_copy, nc.scalar.copy)[ei[0] % 2]

        piece_bounds = [0, 128] + list(range(384, T2, 256)) + [T2]
        pieces_done = [False] * (len(piece_bounds) - 1)

        def qkproj_piece(pi_):
            """q+k projection for one <=256-col piece; fused 1-bank psum."""
            c0, c1 = piece_bounds[pi_], piece_bounds[pi_ + 1]
            W_ = c1 - c0
            for hp in range(2):
                ps = psum.tile([128, 2, 256], F32, tag="misc", bufs=2,
                               name=f"pp{pi_}_{hp}")
                for qk in range(2):
                    # q and k halves share one psum bank: a second start=True
                    # would zero the bank on HW and wipe the q result, so the
                    # whole (qk, dc) sweep is ONE accumulation group
                    for dc in range(D // 128):
                        nc.tensor.matmul(
                            ps[:, qk, :W_],
                            lhsT=wqk(dc, hp * 256 + qk * 128,
                                     hp * 256 + (qk + 1) * 128),
                            rhs=xtc(dc, c0, c1),
                            start=(qk == 0 and dc == 0),
                            stop=(qk == 1 and dc == D // 128 - 1),
                        )
                alt()(qk2[:, :, hp, c0:c1], ps[:, :, :W_])

        def need_pieces(col_end):
            col_end = min(col_end, T2)
            for pi_ in range(len(pieces_done)):
                if piece_bounds[pi_] < col_end and not pieces_done[pi_]:
                    pieces_done[pi_] = True
                    qkproj_piece(pi_)

        def vproj(r):
            """v projection for one slot; kb windows extended to 128 so a
            single full copy is fully initialized (extra rows unused)."""
            s = slots[r]
            nkb = len(s["kbs"])
            psv = psum.tile([128, 2, 256], F32, tag="misc", bufs=2,
                            name=f"psv{r}")
            for j, (k0, nk) in enumerate(s["kbs"]):
                kw = min(128, T2 - k0)
                for dc in range(D // 128):
                    # the two kb halves share a psum bank: one accumulation
                    # group (start zeroes the bank once, up front)
                    nc.tensor.matmul(
                        psv[:kw, j, :],
                        lhsT=xtc(dc, k0, k0 + kw),
                        rhs=wqkv[:, dc, 512:768],
                        start=(dc == 0 and j == 0),
                        stop=(dc == D // 128 - 1 and j == len(s["kbs"]) - 1),
                    )
            kb0 = kb_of_slot[r]
            alt()(v_sb[:, kb0:kb0 + nkb, :, 0:64],
                  psv[:, :nkb, :].rearrange("p k (h d) -> p k h d", h=LH))

        def stage_a(i):
            """scores + exp for qsub unit i (all key blocks, kb j at column
            offset 128j of one 2-bank tile).

            tile_position may not change inside a PSUM accumulation group on
            HW, so heads are grouped by base-partition parity with each
            parity group in its OWN bank (par stride = 2KB)."""
            r, (q0, qw) = units[i]
            kbs = slots[r]["kbs"]
            s_ps = psum.tile([128, 2, 2, 256], F32, tag="sps", bufs=2,
                             name=f"sps{i}")
            for par in range(2):
                # key windows extended to 128 (cost scales with the moving
                # dim only) so s_ps is fully initialized; rows nk:128 unused
                pb = 64 * par
                for j, (k0, nk) in enumerate(kbs):
                    for hi, h in enumerate((par, 2 + par)):
                        nc.tensor.matmul(
                            s_ps[:, par, hi, 128 * j:128 * j + qw],
                            lhsT=qk2[pb:pb + 64, 1, h // 2, k0:k0 + 128],
                            rhs=qk2[pb:pb + 64, 0, h // 2, q0:q0 + qw],
                            start=(j == 0 and hi == 0),
                            stop=(j == len(kbs) - 1 and hi == 1),
                        )
            p1 = p1pool.tile([128, 2, 2, 256], BF, tag="p1", bufs=4,
                             name=f"p1_{i}")
            # single exp per qsub via a 5-dim AP: (par, hi, kb, q)
            s5 = s_ps.rearrange("p a b (c q) -> p a b c q", c=2)
            p5 = p1.rearrange("p a b (c q) -> p a b c q", c=2)
            nc.scalar.activation(
                p5[:, :, :, 0:len(kbs), :qw], s5[:, :, :, 0:len(kbs), :qw],
                mybir.ActivationFunctionType.Exp, scale=0.125,
            )
            return p1

        def stage_b(i, p1s_i):
            """PV + recip + normalize-muls for qsub unit i."""
            r, (q0, qw) = units[i]
            s = slots[r]
            kb0 = kb_of_slot[r]
            nkb = len(s["kbs"])
            oaug = psum.tile([128, LH, 128], F32, tag="oaug", bufs=2,
                             name=f"oaug{i}")
            for j, (k0, nk) in enumerate(s["kbs"]):
                for h in range(LH):
                    nc.tensor.matmul(
                        oaug[:, h, :qw],
                        lhsT=v_sb[0:nk, kb0 + j, h, :],
                        rhs=p1s_i[:nk, h % 2, h // 2, 128 * j:128 * j + qw],
                        start=(j == 0 and h == 0),
                        stop=(j == nkb - 1 and h == LH - 1),
                    )
            recip = rbpool.tile([64, LH, 128], F32, tag="rb", name=f"rc{i}")
            nc.vector.reciprocal(recip[:, :, :qw], oaug[64:128, :, :qw])
            # two muls via strided head reads: even heads land on partitions
            # 0:64 of both head-pair planes, odd heads on 64:128 — exactly the
            # [hd-pair, hp] layout the 256-contraction out-proj wants
            for par in range(2):
                nc.vector.tensor_mul(
                    o_sc[64 * par:64 * par + 64, :, q0:q0 + qw],
                    oaug[0:64, par::2, :qw],
                    recip[:, par::2, :qw],
                )

        def stage_c(oi):
            """output projection + store for one flat 128-grid chunk."""
            g0, gw = ochunks[oi]
            po = psum.tile([128, 4, 128], F32, tag="misc", bufs=2,
                           name=f"po{oi}")
            for doc in range(4):
                for dhc in range(2):
                    nc.tensor.matmul(
                        po[:, doc, :gw],
                        lhsT=wo[:, dhc, doc * 128:(doc + 1) * 128],
                        rhs=o_sc[:, dhc, g0:g0 + gw],
                        start=(doc == 0 and dhc == 0),
                        stop=(doc == 3 and dhc == 1),
                    )
            ost = rbpool.tile([128, 4, 128], F32, tag="ost", name=f"ost{oi}")
            nc.scalar.copy(ost[:, :, :gw], po[:, :, :gw])
            nc.sync.dma_start(out_r[:, :, g0:g0 + gw], ost[:, :, :gw])

        # flat qsub unit list; vproj(r) runs with the LAST qsub of slot r
        # (its PV is still >=1 stage away in the software pipeline)
        units = []
        pre = []   # slot index whose vproj runs before this unit, or None
        for r, s in enumerate(slots):
            for qi, q in enumerate(s["qs"]):
                units.append((r, q))
                pre.append(r if qi == len(s["qs"]) - 1 else None)

        # software pipeline: A(i) one qsub ahead of B(i-1); out-proj grid
        # chunks trail the B covering them by one stage
        NU = len(units)
        p1s = [None] * NU
        last_of_slot = {}
        for i, (r, _) in enumerate(units):
            last_of_slot[r] = i
        cmax = []          # per ochunk: last unit index covering it
        for g0, gw in ochunks:
            cmax.append(max(i for i, (r, (q0, qw)) in enumerate(units)
                            if q0 < g0 + gw and q0 + qw > g0))
        bdone_at = {}
        cqi = 0
        bcount = 0

        def after_b(j):
            nonlocal bcount, cqi
            bcount += 1
            bdone_at[j] = bcount
            while cqi < len(ochunks) and (
                    bdone_at.get(cmax[cqi], 1 << 30)
                    + (0 if cmax[cqi] >= NU - 2 else 1) <= bcount):
                stage_c(cqi)
                cqi += 1

        TRAIL = 3
        for i in range(NU):
            r, (q0, qw) = units[i]
            s = slots[r]
            # key windows reach <=128 cols past each kb start
            need_pieces(max(k0 + 128 for k0, _ in s["kbs"]))
            if pre[i] is not None:
                vproj(pre[i])
            if i >= TRAIL:
                stage_b(i - TRAIL, p1s[i - TRAIL])
                p1s[i - TRAIL] = None
            p1s[i] = stage_a(i)
            if i >= TRAIL:
                after_b(i - TRAIL)
        for i in range(max(0, NU - TRAIL), NU):
            stage_b(i, p1s[i])
            after_b(i)
        while cqi < len(ochunks):
            stage_c(cqi)
            cqi += 1


def build_nc(T2, slots, chunks, ochunks):
    nc = bacc.Bacc("TRN2", target_bir_lowering=False, debug=False, num_devices=8)
    with tile.TileContext(nc) as tc:
        _kernel_body(tc, T2, slots, chunks, ochunks)
    nc.compile()
    return nc


def prepare(X, Wc, bc, Win, Wout):
    """Host-side clustering, canonical layout, and per-core input maps."""
    X = np.asarray(X, np.float32)
    Wc = np.asarray(Wc, np.float32)
    bc = np.asarray(bc, np.float32)
    Win = np.asarray(Win, np.float32)
    Wout = np.asarray(Wout, np.float32)

    assign_all = np.stack([(X[b] @ Wc.T + bc).argmax(-1) for b in range(B)])
    T2, slots, chunks, ochunks, order, W = make_schedule(assign_all)

    per_batch = []
    poss = []
    for b in range(B):
        a = assign_all[b]
        X2 = np.zeros((T2, D), np.float32)
        pos = np.empty(T, np.int64)
        for r in range(C):
            c = order[b, r]
            toks = np.nonzero(a == c)[0]
            n = len(toks)
            A0 = slots[r]["a"] + 1   # col A0-1 is the denominator pad column
            X2[A0:A0 + n] = X[b, toks]
            pos[toks] = np.arange(A0, A0 + n)
        per_batch.append({"xt": np.ascontiguousarray(X2.T).astype(NPBF)})
        poss.append(pos)

    per_half = []
    for hh in range(2):
        r = slice(hh * 256, (hh + 1) * 256)
        wq, wk, wv = Win[0:D][r].T, Win[D:2 * D][r].T, Win[2 * D:][r].T
        # [q-hp0 | k-hp0 | q-hp1 | k-hp1 | v]: hp0's q+k contiguous so the
        # first DMA wave is a single transfer
        wqkv = np.concatenate(
            [wq[:, 0:128], wk[:, 0:128], wq[:, 128:256], wk[:, 128:256], wv],
            axis=1,
        )
        per_half.append({
            "wqkv": np.ascontiguousarray(wqkv).astype(NPBF),
            "wo": np.ascontiguousarray(Wout[:, r].T).astype(NPBF),
        })

    in_maps = []
    for g in range(8):
        m = dict(per_batch[g // 2], **per_half[g % 2])
        # packed wave-1 tensor: [xt cols 0:128 | q/k weight cols 0:512]
        m["w1"] = np.ascontiguousarray(
            np.concatenate([m["xt"][:, 0:128], m["wqkv"][:, 0:512]], axis=1))
        in_maps.append(m)
    return (T2, slots, chunks, ochunks), in_maps, poss


_NC_CACHE = {}


def kernel(X, Wc, bc, Win, bin_, Wout, bout):
    assert not np.any(np.asarray(bin_)), "kernel assumes zero in_proj bias"
    sched, in_maps, poss = prepare(X, Wc, bc, Win, Wout)
    key = (sched[0], _hashable(sched[1]), tuple(sched[2]), tuple(sched[3]))
    if key not in _NC_CACHE:
        _NC_CACHE[key] = build_nc(*sched)
    nc = _NC_CACHE[key]
    res = run_bass_kernel_spmd(nc, in_maps, core_ids=list(range(8)))
    outs = res.results
    bout = np.asarray(bout, np.float32)
    out = np.empty((B, T, D), np.float32)
    for b in range(B):
        full = outs[2 * b]["outT"] + outs[2 * b + 1]["outT"]   # [D, T2]
        out[b] = full.T[poss[b]] + bout
    return out


# revision 96
# speedup vs baseline: 1.0058x; 1.0026x over previous
"""Trainium2 Bass kernel for NeuralClusteringAttention (v2, all-bf16).

Problem: B=4, T=1024, D=512, C=8 clusters, H=8 heads, fp32 reference.
Reference collapses to ONE masked-attention pass (each token belongs to
exactly one cluster; in_proj bias is zero):
    out_i = Wout @ (sum_{j in c_i} e^{s_ij} v_j) / (sum_{j in c_i} e^{s_ij}
            + (T - n_{c_i})) + bout,   s_ij = q_i.k_j / 8

Design (55.1us baseline -> 38.1us):
- Tokens sorted by cluster into canonical slots (width = max over batches +1,
  even; slot order tuned for pipeline ramp/drain). Attention runs on EXACT
  per-slot windows: query chunks are slot sub-ranges, key blocks <=128-wide
  sub-windows of the same slot at arbitrary column offsets. No cross-cluster
  coverage -> no mask rows (contraction 64), ~2.3x less score/exp/PV work
  than block-aligned coverage.
- Softmax denominator: pad columns inside a slot have k=v=0 so they add
  exp(0)=1 each; the slot's first pad column carries M = T - w + 1 in its
  v_sb ones-columns (Pool-engine memset patch), making the total exactly
  sum e^s + (T - n) with no extra compute pass.
- Everything bf16 on the PE (1.0 cycles/row at ANY moving width enables the
  exact-fit narrow matmuls; fp8 empirically fails the 2e-2 gate). bf16
  halves input DMA; output stays fp32 through an SBUF bounce (PSUM cannot
  be DMA'd). Few, large DMAs: each dma_start costs ~630ns serialized on the
  single HWDGE issue device.
- Normalization: DVE reciprocal of the replicated mass rows, then two
  strided-head tensor_muls per qsub writing the [hd-pair, hp] o_sc layout
  the 256-contraction output projection wants.
- Software-pipelined emission (engines execute in order): scores/exp run
  TRAIL=3 units ahead of PV/normalize; out-projection chunks trail their
  covering slots; six zero matmuls warm the PE p-state during the first
  DMA wave; wave 1 is one host-packed [xt-cols-0:128 | q/k-weights] tensor
  (one ~630ns HWDGE issue instead of two); a 44-col trailing qsub on the
  last slot shortens the pipeline drain.
- HW constraints found the hard way: tile_position must not change inside a
  PSUM accumulation group (scores are grouped by base-partition parity with
  each parity group in its own bank); one start=True per psum-bank lifetime
  (a second start zeroes the whole bank); SBUF APs start only at partition
  0/32/64/96; Pool/GPSIMD cannot touch PSUM.

Sharding: 8 cores = (4 batches) x (2 head-halves of 4 heads each); host
sums the two partial output projections per batch and adds bout.
"""

import ml_dtypes
import numpy as np

import concourse.bacc as bacc
import concourse.bass as bass
import concourse.mybir as mybir
import concourse.tile as tile
from concourse.bass_utils import run_bass_kernel_spmd

B, T, D, C, H = 4, 1024, 512, 8, 8
HD = D // H          # 64
LH = 4               # local heads per core
BF = mybir.dt.bfloat16
F32 = mybir.dt.float32
NPBF = ml_dtypes.bfloat16


def make_schedule(assign_all):
    """Canonical slot layout shared by all batches (single SPMD program).

    Returns (T2, slots, chunks, order, W):
      slots: per slot dict(a, w, kbs=[(k0, nk)..], qs=[(q0, qw)..], M)
      chunks: [(c0, c1)] qk-projection column chunks (pairs of slots)
    """
    counts = np.stack([np.bincount(a, minlength=C) for a in assign_all])
    order = np.argsort(-counts, axis=1, kind="stable")   # [B, C] rank->cluster
    sizes = -np.sort(-counts, axis=1)
    W = sizes.max(axis=0) + 1                            # >=1 pad col per slot
    W = (W + 1) // 2 * 2                                 # even widths
    # processing order = column order: ramp in on a mid slot, biggest slots
    # mid-stream, smallest last (short pipeline drain)
    perm = [int(p) for p in np.argsort(-W, kind="stable")]
    perm = perm[4:5] + perm[0:4] + perm[5:]
    W = W[perm]
    order = order[:, perm]
    A = np.zeros(C + 1, np.int64)
    A[1:] = np.cumsum(W)
    T2 = int(A[-1])

    slots = []
    for r in range(C):
        a, w = int(A[r]), int(W[r])
        kbs = [(a, min(128, w))]
        if w > 128:
            kbs.append((a + 128, w - 128))
        T2 = max(T2, a + 128, a + w)
        if w > 128:
            h1 = (w // 2 + 1) // 2 * 2                   # balanced even split
            qs = [(a, h1), (a + h1, w - h1)]
        elif r == C - 1:
            qs = [(a, w - 44), (a + w - 44, 44)]         # short drain chain
        else:
            qs = [(a, w)]
        slots.append(dict(a=a, w=w, kbs=kbs, qs=qs, M=float(T - w + 1)))
    chunks = [(int(A[i]), int(A[min(i + 2, C)])) for i in range(0, C, 2)]
    chunks[-1] = (chunks[-1][0], T2)
    # out-projection chunks: flat 128-grid over the used columns (bank-exact
    # PSUM tiles, 512B DMA elems), independent of slot boundaries
    used = int(A[-1])
    ochunks = [(g, min(g + 128, used) - g) for g in range(0, used, 128)]
    return T2, slots, chunks, ochunks, order, W


def _hashable(slots):
    return tuple(
        (s["a"], s["w"], tuple(s["kbs"]), tuple(s["qs"]), s["M"])
        for s in slots
    )


def _kernel_body(tc, T2, slots, chunks, ochunks):
    nc = tc.nc
    NKB = sum(len(s["kbs"]) for s in slots)

    xt_d = nc.dram_tensor("xt", [D, T2], BF, kind="ExternalInput").ap()
    w1_d = nc.dram_tensor("w1", [D, 640], BF, kind="ExternalInput").ap()
    wqkv_d = nc.dram_tensor("wqkv", [D, 768], BF, kind="ExternalInput").ap()
    wo_d = nc.dram_tensor("wo", [256, D], BF, kind="ExternalInput").ap()
    out_d = nc.dram_tensor("outT", [D, T2], F32, kind="ExternalOutput").ap()
    out_r = out_d.rearrange("(c p) t -> p c t", p=128)

    with (
        tc.tile_pool(name="const", bufs=1) as const,
        tc.tile_pool(name="p1", bufs=4) as p1pool,
        tc.tile_pool(name="rb", bufs=4) as rbpool,
        tc.tile_pool(name="psum", bufs=1, space="PSUM") as psum,
    ):
        # ---- persistent SBUF tiles + input DMAs ----
        # first pieces sized for the earliest possible first matmul: q/k
        # weights, then xt chunk 0, then the rest
        wqkv = const.tile([128, D // 128, 768], BF)
        wqkv_r = wqkv_d.rearrange("(c p) n -> p c n", p=128)
        xt = const.tile([128, D // 128, T2], BF)
        xt_r = xt_d.rearrange("(c p) t -> p c t", p=128)
        # DMA waves, smallest-first. Wave 1 is ONE transfer of a host-packed
        # tensor [xt cols 0:128 | q/k weights] so the first projection piece
        # starts ~1us earlier (each dma_start serializes ~630ns on the single
        # HWDGE issue device, and transfers serialize on DMA_ENGINES)
        w1 = const.tile([128, D // 128, 640], BF)
        w1_r = w1_d.rearrange("(c p) n -> p c n", p=128)
        nc.sync.dma_start(w1[:], w1_r)
        nc.scalar.dma_start(xt[:, :, 128:512], xt_r[:, :, 128:512])
        nc.sync.dma_start(wqkv[:, :, 512:768], wqkv_r[:, :, 512:768])
        nc.scalar.dma_start(xt[:, :, 512:704], xt_r[:, :, 512:704])
        nc.scalar.dma_start(xt[:, :, 704:896], xt_r[:, :, 704:896])
        nc.scalar.dma_start(xt[:, :, 896:T2], xt_r[:, :, 896:T2])
        wo = const.tile([128, 2, D], BF)
        nc.sync.dma_start(wo[:], wo_d.rearrange("(c p) n -> p c n", p=128))

        def xtc(dc, c0, c1):
            # xt columns 0:128 live only in the packed wave-1 tile
            if c1 <= 128:
                return w1[:, dc, c0:c1]
            assert c0 >= 128
            return xt[:, dc, c0:c1]

        def wqk(dc, c0, c1):
            # q/k weight cols 0:512 live only in the packed wave-1 tile
            assert c1 <= 512
            return w1[:, dc, 128 + c0:128 + c1]

        warm = const.tile([128, 512], BF)
        nc.vector.memset(warm, 0.0)
        wps = psum.tile([128, 512], F32, tag="misc", bufs=2, name="wps")
        for wi in range(6):
            nc.tensor.matmul(wps[:, :512], lhsT=warm[:, 0:128], rhs=warm,
                             start=(wi == 0), stop=(wi == 5))

        qk2 = const.tile([128, 2, 2, T2], BF)    # [(q|k), hp, col]
        o_sc = const.tile([128, 2, T2], BF)
        # V augmented with ones-columns (softmax mass lands on psum rows
        # 64:128); the denominator pad column of each slot carries M instead.
        v_sb = const.tile([128, NKB, LH, 128], BF)
        nc.gpsimd.memset(v_sb[:, :, :, 64:128], 1.0)
        kbi = 0
        kb_of_slot = []
        for s in slots:
            kb_of_slot.append(kbi)
            # denominator pad column = FIRST column of the slot (partition 0
            # of kb0 — SBUF APs may only start at partition 0/32/64/96);
            # SBUF-only writes can go on the otherwise-idle Pool engine
            nc.gpsimd.memset(v_sb[0:1, kbi, :, 64:128], s["M"])
            kbi += len(s["kbs"])

        # ---- per-chunk: qk projection, then per-slot v + attention ----
        # ---- emission helpers (software-pipelined schedule below) ----
        ei = [0]

        def alt():
            ei[0] += 1
            return (nc.vector.tensor_copy, nc.scalar.copy)[ei[0] % 2]

        piece_bounds = [0, 128] + list(range(384, T2, 256)) + [T2]
        pieces_done = [False] * (len(piece_bounds) - 1)

        def qkproj_piece(pi_):
            """q+k projection for one <=256-col piece; fused 1-bank psum."""
            c0, c1 = piece_bounds[pi_], piece_bounds[pi_ + 1]
            W_ = c1 - c0
            for hp in range(2):
                ps = psum.tile([128, 2, 256], F32, tag="misc", bufs=2,
                               name=f"pp{pi_}_{hp}")
                for qk in range(2):
                    # q and k halves share one psum bank: a second start=True
                    # would zero the bank on HW and wipe the q result, so the
                    # whole (qk, dc) sweep is ONE accumulation group
                    for dc in range(D // 128):
                        nc.tensor.matmul(
                            ps[:, qk, :W_],
                            lhsT=wqk(dc, hp * 256 + qk * 128,
                                     hp * 256 + (qk + 1) * 128),
                            rhs=xtc(dc, c0, c1),
                            start=(qk == 0 and dc == 0),
                            stop=(qk == 1 and dc == D // 128 - 1),
                        )
                alt()(qk2[:, :, hp, c0:c1], ps[:, :, :W_])

        def need_pieces(col_end):
            col_end = min(col_end, T2)
            for pi_ in range(len(pieces_done)):
                if piece_bounds[pi_] < col_end and not pieces_done[pi_]:
                    pieces_done[pi_] = True
                    qkproj_piece(pi_)

        def vproj(r):
            """v projection for one slot; kb windows extended to 128 so a
            single full copy is fully initialized (extra rows unused)."""
            s = slots[r]
            nkb = len(s["kbs"])
            psv = psum.tile([128, 2, 256], F32, tag="misc", bufs=2,
                            name=f"psv{r}")
            for j, (k0, nk) in enumerate(s["kbs"]):
                kw = min(128, T2 - k0)
                for dc in range(D // 128):
                    # the two kb halves share a psum bank: one accumulation
                    # group (start zeroes the bank once, up front)
                    nc.tensor.matmul(
                        psv[:kw, j, :],
                        lhsT=xtc(dc, k0, k0 + kw),
                        rhs=wqkv[:, dc, 512:768],
                        start=(dc == 0 and j == 0),
                        stop=(dc == D // 128 - 1 and j == len(s["kbs"]) - 1),
                    )
            kb0 = kb_of_slot[r]
            alt()(v_sb[:, kb0:kb0 + nkb, :, 0:64],
                  psv[:, :nkb, :].rearrange("p k (h d) -> p k h d", h=LH))

        def stage_a(i):
            """scores + exp for qsub unit i (all key blocks, kb j at column
            offset 128j of one 2-bank tile).

            tile_position may not change inside a PSUM accumulation group on
            HW, so heads are grouped by base-partition parity with each
            parity group in its OWN bank (par stride = 2KB)."""
            r, (q0, qw) = units[i]
            kbs = slots[r]["kbs"]
            s_ps = psum.tile([128, 2, 2, 256], F32, tag="sps", bufs=2,
                             name=f"sps{i}")
            for par in range(2):
                # key windows extended to 128 (cost scales with the moving
                # dim only) so s_ps is fully initialized; rows nk:128 unused
                pb = 64 * par
                for j, (k0, nk) in enumerate(kbs):
                    for hi, h in enumerate((par, 2 + par)):
                        nc.tensor.matmul(
                            s_ps[:, par, hi, 128 * j:128 * j + qw],
                            lhsT=qk2[pb:pb + 64, 1, h // 2, k0:k0 + 128],
                            rhs=qk2[pb:pb + 64, 0, h // 2, q0:q0 + qw],
                            start=(j == 0 and hi == 0),
                            stop=(j == len(kbs) - 1 and hi == 1),
                        )
            p1 = p1pool.tile([128, 2, 2, 256], BF, tag="p1", bufs=4,
                             name=f"p1_{i}")
            # single exp per qsub via a 5-dim AP: (par, hi, kb, q)
            s5 = s_ps.rearrange("p a b (c q) -> p a b c q", c=2)
            p5 = p1.rearrange("p a b (c q) -> p a b c q", c=2)
            nc.scalar.activation(
                p5[:, :, :, 0:len(kbs), :qw], s5[:, :, :, 0:len(kbs), :qw],
                mybir.ActivationFunctionType.Exp, scale=0.125,
            )
            return p1

        def stage_b(i, p1s_i):
            """PV + recip + normalize-muls for qsub unit i."""
            r, (q0, qw) = units[i]
            s = slots[r]
            kb0 = kb_of_slot[r]
            nkb = len(s["kbs"])
            oaug = psum.tile([128, LH, 128], F32, tag="oaug", bufs=2,
                             name=f"oaug{i}")
            for j, (k0, nk) in enumerate(s["kbs"]):
                for h in range(LH):
                    nc.tensor.matmul(
                        oaug[:, h, :qw],
                        lhsT=v_sb[0:nk, kb0 + j, h, :],
                        rhs=p1s_i[:nk, h % 2, h // 2, 128 * j:128 * j + qw],
                        start=(j == 0 and h == 0),
                        stop=(j == nkb - 1 and h == LH - 1),
                    )
            recip = rbpool.tile([64, LH, 128], F32, tag="rb", name=f"rc{i}")
            nc.vector.reciprocal(recip[:, :, :qw], oaug[64:128, :, :qw])
            # two muls via strided head reads: even heads land on partitions
            # 0:64 of both head-pair planes, odd heads on 64:128 — exactly the
            # [hd-pair, hp] layout the 256-contraction out-proj wants
            for par in range(2):
                nc.vector.tensor_mul(
                    o_sc[64 * par:64 * par + 64, :, q0:q0 + qw],
                    oaug[0:64, par::2, :qw],
                    recip[:, par::2, :qw],
                )

        def stage_c(oi):
            """output projection + store for one flat 128-grid chunk."""
            g0, gw = ochunks[oi]
            po = psum.tile([128, 4, 128], F32, tag="misc", bufs=2,
                           name=f"po{oi}")
            for doc in range(4):
                for dhc in range(2):
                    nc.tensor.matmul(
                        po[:, doc, :gw],
                        lhsT=wo[:, dhc, doc * 128:(doc + 1) * 128],
                        rhs=o_sc[:, dhc, g0:g0 + gw],
                        start=(doc == 0 and dhc == 0),
                        stop=(doc == 3 and dhc == 1),
                    )
            ost = rbpool.tile([128, 4, 128], F32, tag="ost", name=f"ost{oi}")
            nc.scalar.copy(ost[:, :, :gw], po[:, :, :gw])
            nc.sync.dma_start(out_r[:, :, g0:g0 + gw], ost[:, :, :gw])

        # flat qsub unit list; vproj(r) runs with the LAST qsub of slot r
        # (its PV is still >=1 stage away in the software pipeline)
        units = []
        pre = []   # slot index whose vproj runs before this unit, or None
        for r, s in enumerate(slots):
            for qi, q in enumerate(s["qs"]):
                units.append((r, q))
                pre.append(r if qi == len(s["qs"]) - 1 else None)

        # software pipeline: A(i) one qsub ahead of B(i-1); out-proj grid
        # chunks trail the B covering them by one stage
        NU = len(units)
        p1s = [None] * NU
        last_of_slot = {}
        for i, (r, _) in enumerate(units):
            last_of_slot[r] = i
        cmax = []          # per ochunk: last unit index covering it
        for g0, gw in ochunks:
            cmax.append(max(i for i, (r, (q0, qw)) in enumerate(units)
                            if q0 < g0 + gw and q0 + qw > g0))
        bdone_at = {}
        cqi = 0
        bcount = 0

        def after_b(j):
            nonlocal bcount, cqi
            bcount += 1
            bdone_at[j] = bcount
            while cqi < len(ochunks) and (
                    bdone_at.get(cmax[cqi], 1 << 30)
                    + (0 if cmax[cqi] >= NU - 2 else 1) <= bcount):
                stage_c(cqi)
                cqi += 1

        TRAIL = 3
        for i in range(NU):
            r, (q0, qw) = units[i]
            s = slots[r]
            # key windows reach <=128 cols past each kb start
            need_pieces(max(k0 + 128 for k0, _ in s["kbs"]))
            if pre[i] is not None:
                vproj(pre[i])
            if i >= TRAIL:
                stage_b(i - TRAIL, p1s[i - TRAIL])
                p1s[i - TRAIL] = None
            p1s[i] = stage_a(i)
            if i >= TRAIL:
                after_b(i - TRAIL)
        for i in range(max(0, NU - TRAIL), NU):
            stage_b(i, p1s[i])
            after_b(i)
        while cqi < len(ochunks):
            stage_c(cqi)
            cqi += 1


def build_nc(T2, slots, chunks, ochunks):
    nc = bacc.Bacc("TRN2", target_bir_lowering=False, debug=False, num_devices=8)
    with tile.TileContext(nc) as tc:
        _kernel_body(tc, T2, slots, chunks, ochunks)
    nc.compile()
    return nc


def prepare(X, Wc, bc, Win, Wout):
    """Host-side clustering, canonical layout, and per-core input maps."""
    X = np.asarray(X, np.float32)
    Wc = np.asarray(Wc, np.float32)
    bc = np.asarray(bc, np.float32)
    Win = np.asarray(Win, np.float32)
    Wout = np.asarray(Wout, np.float32)

    assign_all = np.stack([(X[b] @ Wc.T + bc).argmax(-1) for b in range(B)])
    T2, slots, chunks, ochunks, order, W = make_schedule(assign_all)

    per_batch = []
    poss = []
    for b in range(B):
        a = assign_all[b]
        X2 = np.zeros((T2, D), np.float32)
        pos = np.empty(T, np.int64)
        for r in range(C):
            c = order[b, r]
            toks = np.nonzero(a == c)[0]
            n = len(toks)
            A0 = slots[r]["a"] + 1   # col A0-1 is the denominator pad column
            X2[A0:A0 + n] = X[b, toks]
            pos[toks] = np.arange(A0, A0 + n)
        per_batch.append({"xt": np.ascontiguousarray(X2.T).astype(NPBF)})
        poss.append(pos)

    per_half = []
    for hh in range(2):
        r = slice(hh * 256, (hh + 1) * 256)
        wq, wk, wv = Win[0:D][r].T, Win[D:2 * D][r].T, Win[2 * D:][r].T
        # [q-hp0 | k-hp0 | q-hp1 | k-hp1 | v]: hp0's q+k contiguous so the
        # first DMA wave is a single transfer
        wqkv = np.concatenate(
            [wq[:, 0:128], wk[:, 0:128], wq[:, 128:256], wk[:, 128:256], wv],
            axis=1,
        )
        per_half.append({
            "wqkv": np.ascontiguousarray(wqkv).astype(NPBF),
            "wo": np.ascontiguousarray(Wout[:, r].T).astype(NPBF),
        })

    in_maps = []
    for g in range(8):
        m = dict(per_batch[g // 2], **per_half[g % 2])
        # packed wave-1 tensor: [xt cols 0:128 | q/k weight cols 0:512]
        m["w1"] = np.ascontiguousarray(
            np.concatenate([m["xt"][:, 0:128], m["wqkv"][:, 0:512]], axis=1))
        in_maps.append(m)
    return (T2, slots, chunks, ochunks), in_maps, poss


_NC_CACHE = {}


def kernel(X, Wc, bc, Win, bin_, Wout, bout):
    assert not np.any(np.asarray(bin_)), "kernel assumes zero in_proj bias"
    sched, in_maps, poss = prepare(X, Wc, bc, Win, Wout)
    key = (sched[0], _hashable(sched[1]), tuple(sched[2]), tuple(sched[3]))
    if key not in _NC_CACHE:
        _NC_CACHE[key] = build_nc(*sched)
    nc = _NC_CACHE[key]
    res = run_bass_kernel_spmd(nc, in_maps, core_ids=list(range(8)))
    outs = res.results
    bout = np.asarray(bout, np.float32)
    out = np.empty((B, T, D), np.float32)
    for b in range(B):
        full = outs[2 * b]["outT"] + outs[2 * b + 1]["outT"]   # [D, T2]
        out[b] = full.T[poss[b]] + bout
    return out
